# revision 8
# baseline (speedup 1.0000x reference)
"""GCN encoder (2-layer) on 8 Trainium2 NeuronCores.

Row-parallel sharding: core r owns rows [r*1024, (r+1)*1024) of x / adjacency.

Math (reference):
    a = A + I;  d = rowsum(a)^-1/2;  a_norm = d[:,None] * a * d[None,:]
    h   = relu(a_norm @ (x @ w1.T + b1))
    out = a_norm @ (h @ w2.T + b2)

Device algorithm per core (no rank-dependence inside the NEFF):
    deg_r  = rowsum(A_r) + 1           (PE ones-matmul over A_r^T tiles)
    d_r    = rsqrt(deg_r)              (local slice of d; never all-gathered:
                                        each rank pre-scales its own support
                                        rows before the all-gather, so the
                                        gathered support is fully column-scaled)
    s1s_r  = d_r * (x_r @ w1.T + b1)   -> AllGather -> s1s (8192 x 256, bf16)
    Q1^T   = s1s^T @ A_r^T + diag-term (identity contribution via small
                                        diagonal matmuls: + d_i * s1s[i,:])
    hhat^T = relu(Q1^T)                (h = d_r * hhat, folded downstream)
    s2s_r  = d_r^2 * (hhat_r @ w2.T) + d_r * b2   -> AllGather -> s2s
    out^T  = d_r * (s2s^T @ A_r^T + diag-term)

All big matmuls are bf16 x bf16 with fp32 PSUM accumulation.
A is cast to bf16 and pre-transposed host-side (layout prep only).
Output is produced transposed ([128, 1024] per core) and re-transposed host-side.
"""

import os
import sys

import numpy as np
import ml_dtypes

sys.path.insert(0, "/opt/trn_rl_repo")

BF16 = ml_dtypes.bfloat16

N, F_IN, F_HID, F_OUT = 8192, 512, 256, 128
NCORES = 8
NB = N // NCORES  # 1024 rows per core
P = 128
NT = NB // P      # 8 local row tiles
JT = N // P       # 64 j tiles
JG = 4            # j tiles per at-group  -> 16 groups of [128, 4, 1024]
NG = JT // JG     # 16
SG = 8            # support j-tiles per group -> 8 groups of [128, 8, F]

_cached = {}


def _build_bass():
    import concourse.bacc as bacc
    import concourse.tile as tile
    import concourse.mybir as mybir
    from concourse.alu_op_type import AluOpType

    dt = mybir.dt
    AF = mybir.ActivationFunctionType

    nc = bacc.Bacc(
        "TRN2",
        target_bir_lowering=False,
        debug=False,
        enable_asserts=True,
        num_devices=NCORES,
    )

    # ---- kernel I/O ----
    at_d = nc.dram_tensor("at", [N, NB], dt.bfloat16, kind="ExternalInput")
    xt_d = nc.dram_tensor("xt", [F_IN, NB], dt.bfloat16, kind="ExternalInput")
    w1t_d = nc.dram_tensor("w1t", [F_IN, F_HID], dt.bfloat16, kind="ExternalInput")
    w2t_d = nc.dram_tensor("w2t", [F_HID, F_OUT], dt.bfloat16, kind="ExternalInput")
    b1_d = nc.dram_tensor("b1r", [1, F_HID], dt.float32, kind="ExternalInput")
    b2_d = nc.dram_tensor("b2r", [1, F_OUT], dt.float32, kind="ExternalInput")
    out_d = nc.dram_tensor("out_t", [F_OUT, NB], dt.float32, kind="ExternalOutput")

    # ---- internal DRAM ----
    ag1_in = nc.dram_tensor("ag1_in", [NB, F_HID], dt.bfloat16, kind="Internal")
    ag1_out = nc.dram_tensor(
        "ag1_out", [N, F_HID], dt.bfloat16, kind="Internal", addr_space="Shared"
    )
    ag2_in = nc.dram_tensor("ag2_in", [NB, F_OUT], dt.bfloat16, kind="Internal")
    ag2_out = nc.dram_tensor(
        "ag2_out", [N, F_OUT], dt.bfloat16, kind="Internal", addr_space="Shared"
    )
    d_dram = nc.dram_tensor("d_scratch", [NT, P], dt.float32, kind="Internal")

    ident_d = nc.inline_tensor(np.eye(P, dtype=BF16), name="ident128")

    rg = [list(range(NCORES))]

    with tile.TileContext(nc) as tc:
        with (
            tc.tile_pool(name="p_at", bufs=NG) as p_at,
            tc.tile_pool(name="p_sup", bufs=SG) as p_sup,
            tc.tile_pool(name="p_x", bufs=F_IN // P) as p_x,
            tc.tile_pool(name="p_misc", bufs=1) as p_misc,
            tc.tile_pool(name="p_ps_big", bufs=4, space="PSUM") as pp_big,
            tc.tile_pool(name="p_ps_small", bufs=2, space="PSUM") as pp_small,
            tc.tile_pool(name="p_ps_deg", bufs=2, space="PSUM") as pp_deg,
        ):
            # ---- constants / weights into SBUF ----
            ones_col = p_misc.tile([P, 1], dt.bfloat16, tag="ones_col", name="ones_col")
            nc.gpsimd.memset(ones_col[:], 1.0)
            ones_row_f32 = p_misc.tile([1, P], dt.float32, tag="ones_row", name="ones_row")
            nc.gpsimd.memset(ones_row_f32[:], 1.0)

            ident = p_misc.tile([P, P], dt.bfloat16, tag="ident", name="ident")
            nc.sync.dma_start(ident[:], ident_d[:])

            w1t_sb = p_misc.tile([P, F_IN // P, F_HID], dt.bfloat16, tag="w1t", name="w1t_sb")
            nc.sync.dma_start(
                w1t_sb[:], w1t_d.ap().rearrange("(t p) f -> p t f", p=P)
            )
            w2t_sb = p_misc.tile([P, F_HID // P, F_OUT], dt.bfloat16, tag="w2t", name="w2t_sb")
            nc.sync.dma_start(
                w2t_sb[:], w2t_d.ap().rearrange("(t p) f -> p t f", p=P)
            )
            b1_sb = p_misc.tile([1, F_HID], dt.float32, tag="b1", name="b1_sb")
            nc.sync.dma_start(b1_sb[:], b1_d[:])
            b2_sb = p_misc.tile([1, F_OUT], dt.float32, tag="b2", name="b2_sb")
            nc.sync.dma_start(b2_sb[:], b2_d[:])

            xt_sb = []
            for k in range(F_IN // P):
                t = p_x.tile([P, NB], dt.bfloat16, tag="xt", name="xt_sb")
                nc.sync.dma_start(t[:], xt_d[k * P : (k + 1) * P, :])
                xt_sb.append(t)

            # ---- A^T tiles: 16 groups of [128, 4, 1024] ----
            at_sb = []
            for g in range(NG):
                t = p_at.tile([P, JG, NB], dt.bfloat16, tag="at", name="at_sb")
                nc.sync.dma_start(
                    t[:],
                    at_d[g * JG * P : (g + 1) * JG * P, :].rearrange(
                        "(t p) i -> p t i", p=P
                    ),
                )
                at_sb.append(t)

            # ---- rowsums via ones-matmul: deg[i] = sum_j A^T[j, i] ----
            deg_ps = [pp_deg.tile([1, NB // 2], dt.float32, tag="deg", name="deg") for _ in range(2)]
            for c in range(2):
                for g in range(NG):
                    for t in range(JG):
                        nc.tensor.matmul(
                            deg_ps[c][:],
                            ones_col[:],
                            at_sb[g][:, t, c * (NB // 2) : (c + 1) * (NB // 2)],
                            start=(g == 0 and t == 0),
                            stop=(g == NG - 1 and t == JG - 1),
                        )

            # ---- d = rsqrt(deg + 1); derived layouts ----
            # every engine needs base_partition 0, so each vector gets its own tile
            deg_sb = p_misc.tile([1, NB], dt.float32, tag="deg_sb", name="deg_sb")
            dinv_deg = p_misc.tile([1, NB], dt.float32, tag="dinv_deg", name="dinv_deg")
            drow = p_misc.tile([1, NB], dt.float32, tag="drow", name="drow")
            dinv_row = p_misc.tile([1, NB], dt.float32, tag="dinv_row", name="dinv_row")
            for c in range(2):
                nc.vector.tensor_scalar_add(
                    deg_sb[:, c * (NB // 2) : (c + 1) * (NB // 2)], deg_ps[c][:], 1.0
                )
            nc.vector.reciprocal(dinv_deg[:], deg_sb[:])
            nc.scalar.sqrt(drow[:], dinv_deg[:])
            # 1/d for the layer-2 bias matmul
            nc.vector.reciprocal(dinv_row[:], drow[:])

            # d as per-partition scalars [128, 8]
            nc.sync.dma_start(d_dram.ap(), drow[:])
            dpart = p_misc.tile([P, NT], dt.float32, tag="dpart", name="dpart")
            nc.sync.dma_start(dpart[:], d_dram.ap().rearrange("t p -> p t"))
            dsq_part = p_misc.tile([P, NT], dt.float32, tag="dsq_part", name="dsq_part")
            nc.vector.tensor_mul(dsq_part[:], dpart[:], dpart[:])

            # d broadcast across partitions [128, 1024] via K=1 outer product
            d_bcast = p_misc.tile([P, NB], dt.float32, tag="d_bcast", name="d_bcast")
            for c in range(2):
                ps = pp_small.tile([P, NB // 2], dt.float32, tag="ps_small", name="ps_small")
                nc.tensor.matmul(
                    ps[:],
                    ones_row_f32[:],
                    drow[:, c * (NB // 2) : (c + 1) * (NB // 2)],
                    start=True,
                    stop=True,
                )
                nc.vector.tensor_copy(
                    d_bcast[:, c * (NB // 2) : (c + 1) * (NB // 2)], ps[:]
                )

            # diagonal d tiles (bf16) for the identity contribution
            diag_sb = []
            for t in range(NT):
                dg = p_misc.tile([P, P], dt.bfloat16, tag="diag", bufs=NT, name="diag")
                nc.vector.tensor_scalar_mul(dg[:], ident[:], dpart[:, t : t + 1])
                diag_sb.append(dg)

            # ---- layer-1 local support: s1s = d_i * (x @ w1.T + b1) ----
            s1loc = []
            for m in range(NT):
                ps = pp_small.tile([P, F_HID], dt.float32, tag="ps_small", name="ps_small")
                for k in range(F_IN // P):
                    nc.tensor.matmul(
                        ps[:],
                        xt_sb[k][:, m * P : (m + 1) * P],
                        w1t_sb[:, k, :],
                        start=(k == 0),
                        stop=False,
                    )
                nc.tensor.matmul(
                    ps[:], ones_row_f32[:], b1_sb[:], start=False, stop=True
                )
                t = p_misc.tile([P, F_HID], dt.bfloat16, tag="s1loc", bufs=NT, name="s1loc")
                nc.scalar.activation(
                    t[:], ps[:], mybir.ActivationFunctionType.Copy,
                    scale=dpart[:, m : m + 1],
                )
                s1loc.append(t)
                nc.sync.dma_start(ag1_in[m * P : (m + 1) * P, :], t[:])

            # ---- AllGather layer-1 support ----
            nc.gpsimd.collective_compute(
                "AllGather",
                mybir.AluOpType.bypass,
                replica_groups=rg,
                ins=[ag1_in.ap()],
                outs=[ag1_out.ap()],
            )

            sg1 = []
            for g in range(SG):
                t = p_sup.tile([P, SG, F_HID], dt.bfloat16, tag="sup", name="sup")
                nc.sync.dma_start(
                    t[:],
                    ag1_out[g * SG * P : (g + 1) * SG * P, :].rearrange(
                        "(t p) f -> p t f", p=P
                    ),
                )
                sg1.append(t)

            # ---- layer-1 aggregation: Q1^T[f, i] = sum_j s1s[j, f] A^T[j, i] ----
            HC = NB // 2  # 512-wide i chunks
            q1_ps = [
                [pp_big.tile([P, HC], dt.float32, tag="ps_big", name="ps_big") for _ in range(2)]
                for _ in range(F_HID // P)
            ]
            for g in range(SG):
                for t in range(SG):
                    jt = g * SG + t
                    ag, sub = jt // JG, jt % JG
                    for m in range(F_HID // P):
                        lhsT = sg1[g][:, t, m * P : (m + 1) * P]
                        for c in range(2):
                            nc.tensor.matmul(
                                q1_ps[m][c][:],
                                lhsT,
                                at_sb[ag][:, sub, c * HC : (c + 1) * HC],
                                start=(jt == 0),
                                stop=False,
                            )
            # identity contribution: += d_i * s1s[i, f]
            for m in range(F_HID // P):
                for c in range(2):
                    for k in range(4):
                        t = c * 4 + k
                        nc.tensor.matmul(
                            q1_ps[m][c][:, k * P : (k + 1) * P],
                            s1loc[t][:, m * P : (m + 1) * P],
                            diag_sb[t][:],
                            start=False,
                            stop=(k == 3),
                        )

            # hhat^T = relu(Q1^T), bf16
            hT = []
            for m in range(F_HID // P):
                t = p_misc.tile([P, NB], dt.bfloat16, tag="hT", bufs=F_HID // P, name="hT")
                for c in range(2):
                    nc.scalar.activation(
                        t[:, c * HC : (c + 1) * HC], q1_ps[m][c][:],
                        mybir.ActivationFunctionType.Relu,
                    )
                hT.append(t)

            # ---- layer-2 local support: s2s = d^2 * (hhat @ w2.T) + d * b2 ----
            s2loc = []
            for m in range(NT):
                ps = pp_small.tile([P, F_OUT], dt.float32, tag="ps_small", name="ps_small")
                for k in range(F_HID // P):
                    nc.tensor.matmul(
                        ps[:],
                        hT[k][:, m * P : (m + 1) * P],
                        w2t_sb[:, k, :],
                        start=(k == 0),
                        stop=False,
                    )
                # bias: (1/d_i) * b2, so the d^2 epilogue scale leaves d_i * b2
                nc.tensor.matmul(
                    ps[:],
                    dinv_row[:, m * P : (m + 1) * P],
                    b2_sb[:],
                    start=False,
                    stop=True,
                )
                t = p_misc.tile([P, F_OUT], dt.bfloat16, tag="s2loc", bufs=NT, name="s2loc")
                nc.scalar.activation(
                    t[:], ps[:], mybir.ActivationFunctionType.Copy,
                    scale=dsq_part[:, m : m + 1],
                )
                s2loc.append(t)
                nc.sync.dma_start(ag2_in[m * P : (m + 1) * P, :], t[:])

            # ---- AllGather layer-2 support ----
            nc.gpsimd.collective_compute(
                "AllGather",
                mybir.AluOpType.bypass,
                replica_groups=rg,
                ins=[ag2_in.ap()],
                outs=[ag2_out.ap()],
            )

            sg2 = []
            for g in range(SG):
                t = p_sup.tile([P, SG, F_OUT], dt.bfloat16, tag="sup", name="sup")
                nc.sync.dma_start(
                    t[:],
                    ag2_out[g * SG * P : (g + 1) * SG * P, :].rearrange(
                        "(t p) f -> p t f", p=P
                    ),
                )
                sg2.append(t)

            # ---- layer-2 aggregation + final scale ----
            o_ps = [pp_big.tile([P, HC], dt.float32, tag="ps_big", name="ps_big") for _ in range(2)]
            for g in range(SG):
                for t in range(SG):
                    jt = g * SG + t
                    ag, sub = jt // JG, jt % JG
                    lhsT = sg2[g][:, t, :]
                    for c in range(2):
                        nc.tensor.matmul(
                            o_ps[c][:],
                            lhsT,
                            at_sb[ag][:, sub, c * HC : (c + 1) * HC],
                            start=(jt == 0),
                            stop=False,
                        )
            for c in range(2):
                for k in range(4):
                    t = c * 4 + k
                    nc.tensor.matmul(
                        o_ps[c][:, k * P : (k + 1) * P],
                        s2loc[t][:],
                        diag_sb[t][:],
                        start=False,
                        stop=(k == 3),
                    )

            outT = []
            for c in range(2):
                t = p_misc.tile([P, HC], dt.float32, tag="hT", bufs=2, name="outT")
                nc.vector.tensor_tensor(
                    t[:], o_ps[c][:], d_bcast[:, c * HC : (c + 1) * HC],
                    op=mybir.AluOpType.mult,
                )
                outT.append(t)
                nc.sync.dma_start(out_d[:, c * HC : (c + 1) * HC], t[:])

    nc.compile()
    return nc


def kernel(x, adjacency_matrix, w1, b1, w2, b2):
    from concourse.bass_utils import run_bass_kernel_spmd

    x = np.asarray(x, dtype=np.float32)
    A = np.asarray(adjacency_matrix, dtype=np.float32)
    w1 = np.asarray(w1, dtype=np.float32)
    b1 = np.asarray(b1, dtype=np.float32)
    w2 = np.asarray(w2, dtype=np.float32)
    b2 = np.asarray(b2, dtype=np.float32)

    if "nc" not in _cached:
        _cached["nc"] = _build_bass()
    nc = _cached["nc"]

    w1t = np.ascontiguousarray(w1.T).astype(BF16)
    w2t = np.ascontiguousarray(w2.T).astype(BF16)
    b1r = np.ascontiguousarray(b1[None, :])
    b2r = np.ascontiguousarray(b2[None, :])

    in_maps = []
    for r in range(NCORES):
        rows = slice(r * NB, (r + 1) * NB)
        in_maps.append(
            {
                "at": np.ascontiguousarray(A[rows, :].T.astype(BF16)),
                "xt": np.ascontiguousarray(x[rows, :].T.astype(BF16)),
                "w1t": w1t,
                "w2t": w2t,
                "b1r": b1r,
                "b2r": b2r,
            }
        )

    trace = bool(int(os.environ.get("KERNEL_TRACE", "0")))
    res = run_bass_kernel_spmd(
        nc, in_maps, core_ids=list(range(NCORES)), trace=trace
    )
    if trace and res.exec_time_ns is not None:
        print(f"HW exec time: {res.exec_time_ns} ns")
        _cached["exec_time_ns"] = res.exec_time_ns
        _cached["results_obj"] = res

    out = np.empty((N, F_OUT), dtype=np.float32)
    for r in range(NCORES):
        out[r * NB : (r + 1) * NB, :] = res.results[r]["out_t"].T
    return out


# revision 9
# speedup vs baseline: 658.6056x; 658.6056x over previous
"""GCN encoder (2-layer) on 8 Trainium2 NeuronCores.

Row-parallel sharding: core r owns rows [r*1024, (r+1)*1024) of x / adjacency.

Math (reference):
    a = A + I;  d = rowsum(a)^-1/2;  a_norm = d[:,None] * a * d[None,:]
    h   = relu(a_norm @ (x @ w1.T + b1))
    out = a_norm @ (h @ w2.T + b2)

Device algorithm per core (no rank-dependence inside the NEFF):
    deg_r  = rowsum(A_r) + 1           (PE ones-matmul over A_r^T tiles)
    d_r    = rsqrt(deg_r)              (local slice of d; never all-gathered:
                                        each rank pre-scales its own support
                                        rows before the all-gather, so the
                                        gathered support is fully column-scaled)
    s1s_r  = d_r * (x_r @ w1.T + b1)   -> AllGather -> s1s (8192 x 256, bf16)
    Q1^T   = s1s^T @ A_r^T + diag-term (identity contribution via small
                                        diagonal matmuls: + d_i * s1s[i,:])
    hhat^T = relu(Q1^T)                (h = d_r * hhat, folded downstream)
    s2s_r  = d_r^2 * (hhat_r @ w2.T) + d_r * b2   -> AllGather -> s2s
    out^T  = d_r * (s2s^T @ A_r^T + diag-term)

All big matmuls are bf16 x bf16 with fp32 PSUM accumulation.
A is cast to bf16 and pre-transposed host-side (layout prep only).
Output is produced transposed ([128, 1024] per core) and re-transposed host-side.
"""

import os
import sys

import numpy as np
import ml_dtypes

sys.path.insert(0, "/opt/trn_rl_repo")

BF16 = ml_dtypes.bfloat16

N, F_IN, F_HID, F_OUT = 8192, 512, 256, 128
NCORES = 8
NB = N // NCORES  # 1024 rows per core
P = 128
NT = NB // P      # 8 local row tiles
JT = N // P       # 64 j tiles
JG = 4            # j tiles per at-group  -> 16 groups of [128, 4, 1024]
NG = JT // JG     # 16
SG = 8            # support j-tiles per group -> 8 groups of [128, 8, F]
HC = NB // 2      # 512-wide i chunks

_cached = {}


def _build_bass(reps=1):
    import concourse.bacc as bacc
    import concourse.tile as tile
    import concourse.mybir as mybir

    dt = mybir.dt

    nc = bacc.Bacc(
        "TRN2",
        target_bir_lowering=False,
        debug=False,
        enable_asserts=True,
        num_devices=NCORES,
    )

    # ---- kernel I/O ----
    at_d = nc.dram_tensor("at", [N, NB], dt.bfloat16, kind="ExternalInput")
    xt_d = nc.dram_tensor("xt", [F_IN, NB], dt.bfloat16, kind="ExternalInput")
    w1t_d = nc.dram_tensor("w1t", [F_IN, F_HID], dt.bfloat16, kind="ExternalInput")
    w2t_d = nc.dram_tensor("w2t", [F_HID, F_OUT], dt.bfloat16, kind="ExternalInput")
    b1_d = nc.dram_tensor("b1r", [1, F_HID], dt.float32, kind="ExternalInput")
    b2_d = nc.dram_tensor("b2r", [1, F_OUT], dt.float32, kind="ExternalInput")
    out_d = nc.dram_tensor("out_t", [F_OUT, NB], dt.float32, kind="ExternalOutput")

    # ---- internal DRAM ----
    ag1_in = nc.dram_tensor("ag1_in", [NB, F_HID], dt.bfloat16, kind="Internal")
    ag1_out = nc.dram_tensor(
        "ag1_out", [N, F_HID], dt.bfloat16, kind="Internal", addr_space="Shared"
    )
    ag2_in = nc.dram_tensor("ag2_in", [NB, F_OUT], dt.bfloat16, kind="Internal")
    ag2_out = nc.dram_tensor(
        "ag2_out", [N, F_OUT], dt.bfloat16, kind="Internal", addr_space="Shared"
    )
    d_dram = nc.dram_tensor("d_scratch", [NT, P], dt.float32, kind="Internal")

    ident_d = nc.inline_tensor(np.eye(P, dtype=BF16), name="ident128")

    rg = [list(range(NCORES))]
    io = dict(
        at_d=at_d, xt_d=xt_d, w1t_d=w1t_d, w2t_d=w2t_d, b1_d=b1_d, b2_d=b2_d,
        out_d=out_d, ag1_in=ag1_in, ag1_out=ag1_out, ag2_in=ag2_in,
        ag2_out=ag2_out, d_dram=d_dram, ident_d=ident_d, rg=rg,
    )

    with tile.TileContext(nc) as tc:
        with (
            tc.tile_pool(name="p_at", bufs=NG) as p_at,
            tc.tile_pool(name="p_sup", bufs=SG) as p_sup,
            tc.tile_pool(name="p_x", bufs=F_IN // P) as p_x,
            tc.tile_pool(name="p_misc", bufs=1) as p_misc,
            tc.tile_pool(name="p_ps_big", bufs=4, space="PSUM") as pp_big,
            tc.tile_pool(name="p_ps_small", bufs=2, space="PSUM") as pp_small,
            tc.tile_pool(name="p_ps_deg", bufs=2, space="PSUM") as pp_deg,
        ):
            pools = dict(
                p_at=p_at, p_sup=p_sup, p_x=p_x, p_misc=p_misc,
                pp_big=pp_big, pp_small=pp_small, pp_deg=pp_deg,
            )
            for _ in range(reps):
                _emit_body(nc, mybir, pools, io)

    nc.compile()
    return nc


def _emit_body(nc, mybir, pools, io):
    dt = mybir.dt
    AF = mybir.ActivationFunctionType
    p_at, p_sup, p_x, p_misc = (
        pools["p_at"], pools["p_sup"], pools["p_x"], pools["p_misc"],
    )
    pp_big, pp_small, pp_deg = pools["pp_big"], pools["pp_small"], pools["pp_deg"]
    at_d, xt_d, w1t_d, w2t_d = io["at_d"], io["xt_d"], io["w1t_d"], io["w2t_d"]
    b1_d, b2_d, out_d = io["b1_d"], io["b2_d"], io["out_d"]
    ag1_in, ag1_out, ag2_in, ag2_out = (
        io["ag1_in"], io["ag1_out"], io["ag2_in"], io["ag2_out"],
    )
    d_dram, ident_d, rg = io["d_dram"], io["ident_d"], io["rg"]

    # ---- constants / weights into SBUF ----
    ones_col = p_misc.tile([P, 1], dt.bfloat16, tag="ones_col", name="ones_col")
    nc.gpsimd.memset(ones_col[:], 1.0)
    ones_row_f32 = p_misc.tile([1, P], dt.float32, tag="ones_row", name="ones_row")
    nc.gpsimd.memset(ones_row_f32[:], 1.0)

    ident = p_misc.tile([P, P], dt.bfloat16, tag="ident", name="ident")
    nc.sync.dma_start(ident[:], ident_d[:])

    w1t_sb = p_misc.tile([P, F_IN // P, F_HID], dt.bfloat16, tag="w1t", name="w1t_sb")
    nc.sync.dma_start(w1t_sb[:], w1t_d.ap().rearrange("(t p) f -> p t f", p=P))
    w2t_sb = p_misc.tile([P, F_HID // P, F_OUT], dt.bfloat16, tag="w2t", name="w2t_sb")
    nc.sync.dma_start(w2t_sb[:], w2t_d.ap().rearrange("(t p) f -> p t f", p=P))
    b1_sb = p_misc.tile([1, F_HID], dt.float32, tag="b1", name="b1_sb")
    nc.sync.dma_start(b1_sb[:], b1_d[:])
    b2_sb = p_misc.tile([1, F_OUT], dt.float32, tag="b2", name="b2_sb")
    nc.sync.dma_start(b2_sb[:], b2_d[:])

    xt_sb = []
    for k in range(F_IN // P):
        t = p_x.tile([P, NB], dt.bfloat16, tag="xt", name="xt_sb")
        nc.sync.dma_start(t[:], xt_d[k * P : (k + 1) * P, :])
        xt_sb.append(t)

    # ---- A^T tiles: 16 groups of [128, 4, 1024] ----
    at_sb = []
    for g in range(NG):
        t = p_at.tile([P, JG, NB], dt.bfloat16, tag="at", name="at_sb")
        nc.sync.dma_start(
            t[:],
            at_d[g * JG * P : (g + 1) * JG * P, :].rearrange("(t p) i -> p t i", p=P),
        )
        at_sb.append(t)

    # ---- rowsums via ones-matmul: deg[i] = sum_j A^T[j, i] ----
    deg_ps = [
        pp_deg.tile([1, HC], dt.float32, tag="deg", name="deg") for _ in range(2)
    ]
    for g in range(NG):
        for t in range(JG):
            for c in range(2):
                nc.tensor.matmul(
                    deg_ps[c][:],
                    ones_col[:],
                    at_sb[g][:, t, c * HC : (c + 1) * HC],
                    start=(g == 0 and t == 0),
                    stop=(g == NG - 1 and t == JG - 1),
                )

    # ---- d = rsqrt(deg + 1); derived layouts ----
    deg_sb = p_misc.tile([1, NB], dt.float32, tag="deg_sb", name="deg_sb")
    for c in range(2):
        nc.vector.tensor_scalar_add(
            deg_sb[:, c * HC : (c + 1) * HC], deg_ps[c][:], 1.0
        )
    dinv_deg = p_misc.tile([1, NB], dt.float32, tag="dinv_deg", name="dinv_deg")
    nc.vector.reciprocal(dinv_deg[:], deg_sb[:])
    drow = p_misc.tile([1, NB], dt.float32, tag="drow", name="drow")
    nc.scalar.sqrt(drow[:], dinv_deg[:])
    # 1/d for the layer-2 bias matmul
    dinv_row = p_misc.tile([1, NB], dt.float32, tag="dinv_row", name="dinv_row")
    nc.vector.reciprocal(dinv_row[:], drow[:])

    # d as per-partition scalars [128, 8]
    nc.sync.dma_start(d_dram.ap(), drow[:])
    dpart = p_misc.tile([P, NT], dt.float32, tag="dpart", name="dpart")
    nc.sync.dma_start(dpart[:], d_dram.ap().rearrange("t p -> p t"))
    dsq_part = p_misc.tile([P, NT], dt.float32, tag="dsq_part", name="dsq_part")
    nc.vector.tensor_mul(dsq_part[:], dpart[:], dpart[:])

    # d broadcast across partitions [128, 1024] via K=1 outer product
    d_bcast = p_misc.tile([P, NB], dt.float32, tag="d_bcast", name="d_bcast")
    for c in range(2):
        ps = pp_small.tile([P, HC], dt.float32, tag="ps_small", name="ps_small")
        nc.tensor.matmul(
            ps[:], ones_row_f32[:], drow[:, c * HC : (c + 1) * HC],
            start=True, stop=True,
        )
        nc.vector.tensor_copy(d_bcast[:, c * HC : (c + 1) * HC], ps[:])

    # diagonal d tiles (bf16) for the identity contribution
    diag_sb = []
    for t in range(NT):
        dg = p_misc.tile([P, P], dt.bfloat16, tag="diag", bufs=NT, name="diag")
        nc.vector.tensor_scalar_mul(dg[:], ident[:], dpart[:, t : t + 1])
        diag_sb.append(dg)

    # ---- layer-1 local support: s1s = d_i * (x @ w1.T + b1) ----
    s1loc = []
    for m in range(NT):
        ps = pp_small.tile([P, F_HID], dt.float32, tag="ps_small", name="ps_small")
        for k in range(F_IN // P):
            nc.tensor.matmul(
                ps[:], xt_sb[k][:, m * P : (m + 1) * P], w1t_sb[:, k, :],
                start=(k == 0), stop=False,
            )
        nc.tensor.matmul(ps[:], ones_row_f32[:], b1_sb[:], start=False, stop=True)
        t = p_misc.tile([P, F_HID], dt.bfloat16, tag="s1loc", bufs=NT, name="s1loc")
        nc.scalar.activation(t[:], ps[:], AF.Copy, scale=dpart[:, m : m + 1])
        s1loc.append(t)
        nc.sync.dma_start(ag1_in[m * P : (m + 1) * P, :], t[:])

    # ---- AllGather layer-1 support ----
    nc.gpsimd.collective_compute(
        "AllGather", mybir.AluOpType.bypass, replica_groups=rg,
        ins=[ag1_in.ap()], outs=[ag1_out.ap()],
    )

    sg1 = []
    for g in range(SG):
        t = p_sup.tile([P, SG, F_HID], dt.bfloat16, tag="sup", name="sup")
        nc.sync.dma_start(
            t[:],
            ag1_out[g * SG * P : (g + 1) * SG * P, :].rearrange(
                "(t p) f -> p t f", p=P
            ),
        )
        sg1.append(t)

    # ---- layer-1 aggregation: Q1^T[f, i] = sum_j s1s[j, f] A^T[j, i] ----
    q1_ps = [
        [
            pp_big.tile([P, HC], dt.float32, tag="ps_big", name="ps_big")
            for _ in range(2)
        ]
        for _ in range(F_HID // P)
    ]
    for g in range(SG):
        for t in range(SG):
            jt = g * SG + t
            ag, sub = jt // JG, jt % JG
            for m in range(F_HID // P):
                lhsT = sg1[g][:, t, m * P : (m + 1) * P]
                for c in range(2):
                    nc.tensor.matmul(
                        q1_ps[m][c][:],
                        lhsT,
                        at_sb[ag][:, sub, c * HC : (c + 1) * HC],
                        start=(jt == 0),
                        stop=False,
                    )
    # identity contribution: += d_i * s1s[i, f]
    for m in range(F_HID // P):
        for c in range(2):
            for k in range(4):
                t = c * 4 + k
                nc.tensor.matmul(
                    q1_ps[m][c][:, k * P : (k + 1) * P],
                    s1loc[t][:, m * P : (m + 1) * P],
                    diag_sb[t][:],
                    start=False,
                    stop=(k == 3),
                )

    # hhat^T = relu(Q1^T), bf16
    hT = []
    for m in range(F_HID // P):
        t = p_misc.tile([P, NB], dt.bfloat16, tag="hT", bufs=F_HID // P, name="hT")
        for c in range(2):
            nc.scalar.activation(t[:, c * HC : (c + 1) * HC], q1_ps[m][c][:], AF.Relu)
        hT.append(t)

    # ---- layer-2 local support: s2s = d^2 * (hhat @ w2.T) + d * b2 ----
    s2loc = []
    for m in range(NT):
        ps = pp_small.tile([P, F_OUT], dt.float32, tag="ps_small", name="ps_small")
        for k in range(F_HID // P):
            nc.tensor.matmul(
                ps[:], hT[k][:, m * P : (m + 1) * P], w2t_sb[:, k, :],
                start=(k == 0), stop=False,
            )
        # bias: (1/d_i) * b2, so the d^2 epilogue scale leaves d_i * b2
        nc.tensor.matmul(
            ps[:], dinv_row[:, m * P : (m + 1) * P], b2_sb[:],
            start=False, stop=True,
        )
        t = p_misc.tile([P, F_OUT], dt.bfloat16, tag="s2loc", bufs=NT, name="s2loc")
        nc.scalar.activation(t[:], ps[:], AF.Copy, scale=dsq_part[:, m : m + 1])
        s2loc.append(t)
        nc.sync.dma_start(ag2_in[m * P : (m + 1) * P, :], t[:])

    # ---- AllGather layer-2 support ----
    nc.gpsimd.collective_compute(
        "AllGather", mybir.AluOpType.bypass, replica_groups=rg,
        ins=[ag2_in.ap()], outs=[ag2_out.ap()],
    )

    sg2 = []
    for g in range(SG):
        t = p_sup.tile([P, SG, F_OUT], dt.bfloat16, tag="sup", name="sup")
        nc.sync.dma_start(
            t[:],
            ag2_out[g * SG * P : (g + 1) * SG * P, :].rearrange(
                "(t p) f -> p t f", p=P
            ),
        )
        sg2.append(t)

    # ---- layer-2 aggregation + final scale ----
    o_ps = [
        pp_big.tile([P, HC], dt.float32, tag="ps_big", name="ps_big")
        for _ in range(2)
    ]
    for g in range(SG):
        for t in range(SG):
            jt = g * SG + t
            ag, sub = jt // JG, jt % JG
            lhsT = sg2[g][:, t, :]
            for c in range(2):
                nc.tensor.matmul(
                    o_ps[c][:],
                    lhsT,
                    at_sb[ag][:, sub, c * HC : (c + 1) * HC],
                    start=(jt == 0),
                    stop=False,
                )
    for c in range(2):
        for k in range(4):
            t = c * 4 + k
            nc.tensor.matmul(
                o_ps[c][:, k * P : (k + 1) * P],
                s2loc[t][:],
                diag_sb[t][:],
                start=False,
                stop=(k == 3),
            )

    for c in range(2):
        t = p_misc.tile([P, HC], dt.float32, tag="hT", bufs=2, name="outT")
        nc.vector.tensor_tensor(
            t[:], o_ps[c][:], d_bcast[:, c * HC : (c + 1) * HC],
            op=mybir.AluOpType.mult,
        )
        nc.sync.dma_start(out_d[:, c * HC : (c + 1) * HC], t[:])


def make_in_maps(x, A, w1, b1, w2, b2):
    w1t = np.ascontiguousarray(w1.T).astype(BF16)
    w2t = np.ascontiguousarray(w2.T).astype(BF16)
    b1r = np.ascontiguousarray(b1[None, :]).astype(np.float32)
    b2r = np.ascontiguousarray(b2[None, :]).astype(np.float32)
    in_maps = []
    for r in range(NCORES):
        rows = slice(r * NB, (r + 1) * NB)
        in_maps.append(
            {
                "at": np.ascontiguousarray(A[rows, :].T.astype(BF16)),
                "xt": np.ascontiguousarray(x[rows, :].T.astype(BF16)),
                "w1t": w1t,
                "w2t": w2t,
                "b1r": b1r,
                "b2r": b2r,
            }
        )
    return in_maps


def kernel(x, adjacency_matrix, w1, b1, w2, b2):
    from concourse.bass_utils import run_bass_kernel_spmd

    x = np.asarray(x, dtype=np.float32)
    A = np.asarray(adjacency_matrix, dtype=np.float32)
    w1 = np.asarray(w1, dtype=np.float32)
    b1 = np.asarray(b1, dtype=np.float32)
    w2 = np.asarray(w2, dtype=np.float32)
    b2 = np.asarray(b2, dtype=np.float32)

    if "nc" not in _cached:
        _cached["nc"] = _build_bass()
    nc = _cached["nc"]

    in_maps = make_in_maps(x, A, w1, b1, w2, b2)
    res = run_bass_kernel_spmd(nc, in_maps, core_ids=list(range(NCORES)))

    out = np.empty((N, F_OUT), dtype=np.float32)
    for r in range(NCORES):
        out[r * NB : (r + 1) * NB, :] = res.results[r]["out_t"].T
    return out
